# revision 6
# baseline (speedup 1.0000x reference)
"""Multi-layer GAT (2-layer graph attention network) on 8 Trainium2 NeuronCores.

Sharding: query-node rows of the NxN attention problem are sharded across the
8 cores (512 rows each); weights and the column copy of Wh are replicated.
The only collective is an AllGather of the layer-1 output h [4096, 256]
(transposed, f-major) between the two GAT layers.

Score math per core, computed transposed (j on partitions, i on free dim):
    X[j, i]  = src[i] + dst[j] + M[j, i]          (M = 0 for edge, -1e4 else)
    T[j, i]  = exp(max(X, 0.2 X))                 (exact leaky_relu via DVE)
    out[f,i] = (Wh^T @ T)[f, i] / (ones @ T)[i]   (ones column fused in lhsT)
src/dst/bias terms are folded into an extended weight matrix on the host, so
they fall out of the same matmul that computes Wh.
"""

import numpy as np
import ml_dtypes

import concourse.bacc as bacc
import concourse.mybir as mybir
from concourse.tile import TileContext
from concourse.bass_utils import run_bass_kernel_spmd
from concourse.masks import make_identity

F32 = mybir.dt.float32
BF16 = mybir.dt.bfloat16
AF = mybir.ActivationFunctionType
ALU = mybir.AluOpType

N, NFEAT, NHID, NCLASS, NHEADS = 4096, 512, 64, 40, 4
NCORES = 8
NS = N // NCORES          # 512 rows (query nodes) per core
NT = N // 128             # 32 j-tiles
KX = NFEAT // 128         # 4 k-tiles over input features
C1 = NHEADS * (NHID + 3)  # 268 fused-weight cols, 67 per head: src,dst,Wh[64],ones
C2 = 67                   # src2,dst2,Wo(40)+pad(24),ones — ones at col 66 so denom hits partition 64
ALPHA = 0.2
MASKVAL = -1e4

_compiled = None


def _build():
    nc = bacc.Bacc("TRN2", num_devices=NCORES)

    xT = nc.dram_tensor("xT", [NFEAT, N], F32, kind="ExternalInput")
    xTs = nc.dram_tensor("xTs", [NFEAT, NS], F32, kind="ExternalInput")
    Wb = nc.dram_tensor("Wb", [NFEAT, C1], F32, kind="ExternalInput")
    bb = nc.dram_tensor("bb", [128, C1], F32, kind="ExternalInput")
    Wob = nc.dram_tensor("Wob", [2 * 128, C2], F32, kind="ExternalInput")
    bob = nc.dram_tensor("bob", [128, C2], F32, kind="ExternalInput")
    Mt = nc.dram_tensor("Mt", [N, NS], BF16, kind="ExternalInput")
    y = nc.dram_tensor("y", [NS, NCLASS], F32, kind="ExternalOutput")

    with TileContext(nc) as tc:
        with (
            tc.tile_pool(name="const", bufs=1) as cp,
            tc.tile_pool(name="mask", bufs=1) as mp,
            tc.tile_pool(name="whext", bufs=1) as wp,
            tc.tile_pool(name="h1g", bufs=1) as gp,
            tc.tile_pool(name="work", bufs=2) as wk,
            tc.tile_pool(name="psA", bufs=1, space="PSUM") as psA,
            tc.tile_pool(name="dram", bufs=1, space="DRAM") as dr,
        ):
            # ---- constants / weights
            ident = cp.tile([128, 128], F32, tag="ident", name="ident")
            make_identity(nc, ident[:])
            ones = cp.tile([1, NHID], F32, tag="ones", name="ones")
            nc.vector.memset(ones[:], 1.0)
            wb_t = [cp.tile([128, C1], F32, tag=f"wb{k}", name=f"wb{k}") for k in range(KX)]
            for k in range(KX):
                nc.sync.dma_start(out=wb_t[k][:], in_=Wb[k * 128:(k + 1) * 128, :])
            bb_t = cp.tile([128, C1], F32, tag="bb", name="bb")
            nc.sync.dma_start(out=bb_t[:], in_=bb[:])
            wo_t = [cp.tile([128, C2], F32, tag=f"wo{k}", name=f"wo{k}") for k in range(2)]
            for k in range(2):
                nc.sync.dma_start(out=wo_t[k][:], in_=Wob[k * 128:(k + 1) * 128, :])
            bo_t = cp.tile([128, C2], F32, tag="bo", name="bo")
            nc.sync.dma_start(out=bo_t[:], in_=bob[:])

            # ---- mask tiles (bf16 additive), resident through both layers
            m_t = [mp.tile([128, NS], BF16, tag=f"m{j}", name=f"m{j}") for j in range(NT)]
            for j in range(NT):
                nc.sync.dma_start(out=m_t[j][:], in_=Mt[j * 128:(j + 1) * 128, :])

            # ---- phase B: Wh_ext for all nodes + for own shard
            wh = [wp.tile([128, C1], F32, tag=f"wh{t}", name=f"wh{t}") for t in range(NT)]
            whs = [wp.tile([128, C1], F32, tag=f"whs{t}", name=f"whs{t}") for t in range(4)]
            with tc.tile_pool(name="xpool", bufs=1) as xp:
                for t in range(NT):
                    ps = psA.tile([128, C1], F32, tag="mmps", name="mmps", bufs=2)
                    for k in range(KX):
                        xk = xp.tile([128, 128], F32, tag="xst", name="xst", bufs=8)
                        nc.sync.dma_start(
                            out=xk[:], in_=xT[k * 128:(k + 1) * 128, t * 128:(t + 1) * 128])
                        nc.tensor.matmul(
                            out=ps[:], lhsT=xk[:],
                            rhs=wb_t[k][:], start=(k == 0), stop=(k == KX - 1))
                    nc.vector.tensor_tensor(out=wh[t][:], in0=ps[:], in1=bb_t[:], op=ALU.add)
                for t in range(4):
                    ps = psA.tile([128, C1], F32, tag="mmps", name="mmps", bufs=2)
                    for k in range(KX):
                        xk = xp.tile([128, 128], F32, tag="xst", name="xst", bufs=8)
                        nc.sync.dma_start(
                            out=xk[:], in_=xTs[k * 128:(k + 1) * 128, t * 128:(t + 1) * 128])
                        nc.tensor.matmul(
                            out=ps[:], lhsT=xk[:],
                            rhs=wb_t[k][:], start=(k == 0), stop=(k == KX - 1))
                    nc.vector.tensor_tensor(out=whs[t][:], in0=ps[:], in1=bb_t[:], op=ALU.add)

            # ---- phase C: broadcast src scores across partitions (PE transpose)
            src_b = [cp.tile([128, NS], F32, tag=f"srcb{h}", name=f"srcb{h}") for h in range(NHEADS)]
            for h in range(NHEADS):
                for t in range(4):
                    tp = psA.tile([128, 128], F32, tag="bc", name="bc", bufs=2)
                    nc.tensor.transpose(
                        out=tp[:], in_=whs[t][:, 67 * h:67 * h + 1].to_broadcast([128, 128]),
                        identity=ident[:])
                    nc.scalar.copy(out=src_b[h][:, t * 128:(t + 1) * 128], in_=tp[:])

            h1t = [cp.tile([128, NS], F32, tag=f"h1t{i}", name=f"h1t{i}") for i in range(2)]

            # ---- phase D: layer-1 attention, one head at a time
            for h in range(NHEADS):
                att = psA.tile([NHID + 1, NS], F32, tag="att", name="att", bufs=2)
                for j in range(NT):
                    xt = wk.tile([128, NS], F32, tag="xt", name="xt", bufs=4)
                    nc.vector.scalar_tensor_tensor(
                        out=xt[:], in0=src_b[h][:], scalar=wh[j][:, 67 * h + 1:67 * h + 2],
                        in1=m_t[j][:], op0=ALU.add, op1=ALU.add)
                    sl = wk.tile([128, NS], F32, tag="sl", name="sl", bufs=4)
                    nc.vector.scalar_tensor_tensor(
                        out=sl[:], in0=xt[:], scalar=ALPHA, in1=xt[:],
                        op0=ALU.mult, op1=ALU.max)
                    tt = wk.tile([128, NS], F32, tag="tt", name="tt", bufs=4)
                    nc.scalar.activation(tt[:], sl[:], AF.Exp)
                    nc.tensor.matmul(
                        out=att[:], lhsT=wh[j][:, 67 * h + 2:67 * h + 67], rhs=tt[:],
                        start=(j == 0), stop=(j == NT - 1))
                rec = wk.tile([1, NS], F32, tag="rec", name="rec")
                nc.vector.reciprocal(out=rec[:], in_=att[NHID:NHID + 1, :])
                bcn = psA.tile([NHID, NS], F32, tag="bcn", name="bcn", bufs=1)
                nc.tensor.matmul(out=bcn[:], lhsT=ones[:], rhs=rec[:], start=True, stop=True)
                nsb = wk.tile([NHID, NS], F32, tag="nsb", name="nsb")
                nc.scalar.copy(out=nsb[:], in_=att[0:NHID, :])
                pre = wk.tile([NHID, NS], F32, tag="pre", name="pre")
                nc.vector.tensor_tensor(out=pre[:], in0=nsb[:], in1=bcn[:], op=ALU.mult)
                # elu(x) = min(exp(x),1) - 1 + relu(x)
                ex = wk.tile([NHID, NS], F32, tag="ex", name="ex")
                nc.scalar.activation(ex[:], pre[:], AF.Exp)
                rm1 = wk.tile([NHID, NS], F32, tag="rm1", name="rm1")
                nc.vector.tensor_scalar(out=rm1[:], in0=pre[:], scalar1=0.0, scalar2=-1.0,
                                        op0=ALU.max, op1=ALU.add)
                nc.vector.scalar_tensor_tensor(
                    out=h1t[h // 2][64 * (h % 2):64 * (h % 2) + 64, :],
                    in0=ex[:], scalar=1.0, in1=rm1[:], op0=ALU.min, op1=ALU.add)

            # ---- phase E: AllGather h1^T [256, 512] -> [8*256, 512]
            agin = dr.tile([2 * 128, NS], F32, tag="agin", name="agin")
            agout = dr.tile([NCORES * 2 * 128, NS], F32, tag="agout", name="agout")
            for i in range(2):
                nc.sync.dma_start(out=agin[i * 128:(i + 1) * 128, :], in_=h1t[i][:])
            nc.gpsimd.collective_compute(
                "AllGather", ALU.bypass,
                replica_groups=[list(range(NCORES))],
                ins=[agin[:].opt()],
                outs=[agout[:].opt()])
            h1g = [gp.tile([128, NS], F32, tag=f"g{i}", name=f"g{i}") for i in range(2 * NCORES)]
            for i in range(2 * NCORES):
                nc.sync.dma_start(out=h1g[i][:], in_=agout[i * 128:(i + 1) * 128, :])

            # ---- phase F: Wh2_ext for all nodes + own shard
            wh2 = [wp.tile([128, C2], F32, tag=f"wh2_{t}", name=f"wh2_{t}") for t in range(NT)]
            for t in range(NT):
                r, i0 = t // 4, (t % 4) * 128
                ps = psA.tile([128, C2], F32, tag="mmps", name="mmps", bufs=2)
                for k in range(2):
                    nc.tensor.matmul(out=ps[:], lhsT=h1g[2 * r + k][:, i0:i0 + 128],
                                     rhs=wo_t[k][:], start=(k == 0), stop=(k == 1))
                nc.vector.tensor_tensor(out=wh2[t][:], in0=ps[:], in1=bo_t[:], op=ALU.add)
            whs2 = [wp.tile([128, C2], F32, tag=f"whs2_{t}", name=f"whs2_{t}") for t in range(4)]
            for t in range(4):
                ps = psA.tile([128, C2], F32, tag="mmps", name="mmps", bufs=2)
                for k in range(2):
                    nc.tensor.matmul(out=ps[:], lhsT=h1t[k][:, t * 128:(t + 1) * 128],
                                     rhs=wo_t[k][:], start=(k == 0), stop=(k == 1))
                nc.vector.tensor_tensor(out=whs2[t][:], in0=ps[:], in1=bo_t[:], op=ALU.add)

            # ---- phase G: broadcast src2
            src2_b = cp.tile([128, NS], F32, tag="src2b", name="src2b")
            for t in range(4):
                tp = psA.tile([128, 128], F32, tag="bc", name="bc", bufs=2)
                nc.tensor.transpose(
                    out=tp[:], in_=whs2[t][:, 0:1].to_broadcast([128, 128]),
                    identity=ident[:])
                nc.scalar.copy(out=src2_b[:, t * 128:(t + 1) * 128], in_=tp[:])

            # ---- phase H: layer-2 attention (single head)
            att2 = psA.tile([65, NS], F32, tag="att", name="att2", bufs=2)
            for j in range(NT):
                xt = wk.tile([128, NS], F32, tag="xt", name="xt", bufs=4)
                nc.vector.scalar_tensor_tensor(
                    out=xt[:], in0=src2_b[:], scalar=wh2[j][:, 1:2],
                    in1=m_t[j][:], op0=ALU.add, op1=ALU.add)
                sl = wk.tile([128, NS], F32, tag="sl", name="sl", bufs=4)
                nc.vector.scalar_tensor_tensor(
                    out=sl[:], in0=xt[:], scalar=ALPHA, in1=xt[:],
                    op0=ALU.mult, op1=ALU.max)
                tt = wk.tile([128, NS], F32, tag="tt", name="tt", bufs=4)
                nc.scalar.activation(tt[:], sl[:], AF.Exp)
                nc.tensor.matmul(out=att2[:], lhsT=wh2[j][:, 2:67], rhs=tt[:],
                                 start=(j == 0), stop=(j == NT - 1))
            rec = wk.tile([1, NS], F32, tag="rec", name="rec")
            nc.vector.reciprocal(out=rec[:], in_=att2[64:65, :])
            bcn2 = psA.tile([NCLASS, NS], F32, tag="bcn", name="bcn2", bufs=1)
            nc.tensor.matmul(out=bcn2[:], lhsT=ones[:, 0:NCLASS], rhs=rec[:], start=True, stop=True)
            nsb = wk.tile([NCLASS, NS], F32, tag="nsb", name="nsb")
            nc.scalar.copy(out=nsb[:], in_=att2[0:NCLASS, :])
            o2 = wk.tile([NCLASS, NS], F32, tag="pre", name="o2")
            nc.vector.tensor_tensor(out=o2[:], in0=nsb[:], in1=bcn2[:], op=ALU.mult)
            ex = wk.tile([NCLASS, NS], F32, tag="ex", name="ex2")
            nc.scalar.activation(ex[:], o2[:], AF.Exp)
            rm1 = wk.tile([NCLASS, NS], F32, tag="rm1", name="rm2")
            nc.vector.tensor_scalar(out=rm1[:], in0=o2[:], scalar1=0.0, scalar2=-1.0,
                                    op0=ALU.max, op1=ALU.add)
            o2e = cp.tile([NCLASS, NS], F32, tag="o2e", name="o2e")
            nc.vector.scalar_tensor_tensor(out=o2e[:], in0=ex[:], scalar=1.0, in1=rm1[:],
                                           op0=ALU.min, op1=ALU.add)

            # ---- phase I: transpose back + log_softmax + store
            for c in range(4):
                tp = psA.tile([128, NCLASS], F32, tag="bc", name="tr", bufs=2)
                nc.tensor.transpose(out=tp[:], in_=o2e[:, c * 128:(c + 1) * 128],
                                    identity=ident[0:NCLASS, 0:NCLASS])
                z = wk.tile([128, NCLASS], F32, tag="z", name="z")
                nc.scalar.copy(out=z[:], in_=tp[:])
                m = wk.tile([128, 1], F32, tag="mred", name="mred")
                nc.vector.reduce_max(out=m[:], in_=z[:], axis=mybir.AxisListType.X)
                negm = wk.tile([128, 1], F32, tag="negm", name="negm")
                nc.vector.tensor_scalar_mul(out=negm[:], in0=m[:], scalar1=-1.0)
                ez = wk.tile([128, NCLASS], F32, tag="ez", name="ez")
                ssum = wk.tile([128, 1], F32, tag="ssum", name="ssum")
                nc.scalar.activation(ez[:], z[:], AF.Exp, bias=negm[:, 0:1],
                                     accum_out=ssum[:, 0:1])
                ls = wk.tile([128, 1], F32, tag="ls", name="ls")
                nc.scalar.activation(ls[:], ssum[:], AF.Ln)
                mls = wk.tile([128, 1], F32, tag="mls", name="mls")
                nc.vector.tensor_tensor(out=mls[:], in0=m[:], in1=ls[:], op=ALU.add)
                yt = wk.tile([128, NCLASS], F32, tag="yt", name="yt")
                nc.vector.tensor_scalar(out=yt[:], in0=z[:], scalar1=mls[:, 0:1],
                                        scalar2=None, op0=ALU.subtract)
                nc.sync.dma_start(out=y[c * 128:(c + 1) * 128, :], in_=yt[:])

    nc.compile()
    return nc


def _prep_inputs(x, edge_index, W1, b1, a1, ab1, Wo, bo, ao, abo):
    x = np.asarray(x, np.float32)
    W1 = np.asarray(W1, np.float32)
    b1 = np.asarray(b1, np.float32)
    a1 = np.asarray(a1, np.float32)
    ab1 = np.asarray(ab1, np.float32)
    Wo = np.asarray(Wo, np.float32)
    bo = np.asarray(bo, np.float32)
    ao = np.asarray(ao, np.float32)
    abo = np.asarray(abo, np.float32)

    xT = np.ascontiguousarray(x.T)

    Wb = np.zeros((NFEAT, C1), np.float32)
    bbrow = np.zeros((C1,), np.float32)
    for h in range(NHEADS):
        c = 67 * h
        Wb[:, c + 0] = W1[h] @ a1[h, :NHID]
        Wb[:, c + 1] = W1[h] @ a1[h, NHID:]
        Wb[:, c + 2:c + 2 + NHID] = W1[h]
        bbrow[c + 0] = b1[h] @ a1[h, :NHID]
        bbrow[c + 1] = b1[h] @ a1[h, NHID:] + ab1[h]
        bbrow[c + 2:c + 2 + NHID] = b1[h]
        bbrow[c + 66] = 1.0
    bb = np.broadcast_to(bbrow, (128, C1)).copy()

    Wob = np.zeros((2 * 128, C2), np.float32)
    Wob[:, 0] = Wo @ ao[:NCLASS]
    Wob[:, 1] = Wo @ ao[NCLASS:]
    Wob[:, 2:2 + NCLASS] = Wo
    borow = np.zeros((C2,), np.float32)
    borow[0] = bo @ ao[:NCLASS]
    borow[1] = bo @ ao[NCLASS:] + abo
    borow[2:2 + NCLASS] = bo
    borow[66] = 1.0
    bob = np.broadcast_to(borow, (128, C2)).copy()

    # additive mask, transposed: M[j, i] = 0 if adj[i, j] else -1e4
    ei = np.asarray(edge_index).astype(np.int64)
    Madd = np.full((N, N), MASKVAL, np.float32)
    Madd[ei[1], ei[0]] = 0.0
    Madd = Madd.astype(ml_dtypes.bfloat16)

    in_maps = []
    for c in range(NCORES):
        sl = slice(c * NS, (c + 1) * NS)
        in_maps.append({
            "xT": xT,
            "xTs": np.ascontiguousarray(xT[:, sl]),
            "Wb": Wb, "bb": bb, "Wob": Wob, "bob": bob,
            "Mt": np.ascontiguousarray(Madd[:, sl]),
        })
    return in_maps


def kernel(x, edge_index, W1, b1, a1, ab1, Wo, bo, ao, abo, _trace=False):
    global _compiled
    if _compiled is None:
        _compiled = _build()
    in_maps = _prep_inputs(x, edge_index, W1, b1, a1, ab1, Wo, bo, ao, abo)
    res = run_bass_kernel_spmd(_compiled, in_maps, core_ids=list(range(NCORES)),
                               trace=_trace)
    kernel.last_result = res
    return np.concatenate([res.results[c]["y"] for c in range(NCORES)], axis=0)


# revision 27
# speedup vs baseline: 1123.6602x; 1123.6602x over previous
"""Multi-layer GAT (2-layer graph attention network) on 8 Trainium2 NeuronCores.

Sharding: query-node rows of the NxN attention problem are sharded across the
8 cores (512 rows each); weights and the column copy of Wh are replicated.
The only collective is an AllGather of the layer-1 output h [4096, 256]
(transposed, f-major) between the two GAT layers.

Score math per core, computed transposed (j on partitions, i on free dim):
    X[j, i]  = src[i] + dst[j] + M[j, i]          (M = 0 for edge, -1e4 else)
    T[j, i]  = exp(max(X, 0.2 X))                 (exact leaky_relu via DVE)
    out[f,i] = (Wh^T @ T)[f, i] / (ones @ T)[i]   (ones column fused in lhsT)
src/dst/bias terms are folded into an extended weight matrix on the host, so
they fall out of the same matmul that computes Wh.
"""

import numpy as np
import ml_dtypes

import concourse.bacc as bacc
import concourse.mybir as mybir
from concourse.tile import TileContext
from concourse.bass_utils import run_bass_kernel_spmd
from concourse.masks import make_identity

F32 = mybir.dt.float32
BF16 = mybir.dt.bfloat16
FP16 = mybir.dt.float16
F32R = mybir.dt.float32r
AF = mybir.ActivationFunctionType
ALU = mybir.AluOpType

N, NFEAT, NHID, NCLASS, NHEADS = 4096, 512, 64, 40, 4
NCORES = 8
NS = N // NCORES          # 512 rows (query nodes) per core
NT = N // 128             # 32 j-tiles
KX = NFEAT // 128         # 4 k-tiles over input features
C1 = NHEADS * (NHID + 3)  # 268 fused-weight cols, 67 per head: src,dst,Wh[64],ones
C2 = 67                   # src2,dst2,Wo(40)+pad(24),ones — ones at col 66 so denom hits partition 64
ALPHA = 0.2
MASKVAL = -1e4

_compiled = None


def _build(sim_mode=False, stop_after=None):
    nc = bacc.Bacc("TRN2", num_devices=1 if sim_mode else NCORES)

    xP = nc.dram_tensor("xP", [NT + 4, 128, NFEAT], FP16, kind="ExternalInput")
    Wb = nc.dram_tensor("Wb", [NFEAT, C1], FP16, kind="ExternalInput")
    bb = nc.dram_tensor("bb", [1, C1], FP16, kind="ExternalInput")
    Wob = nc.dram_tensor("Wob", [2 * 128, C2], FP16, kind="ExternalInput")
    bob = nc.dram_tensor("bob", [1, C2], FP16, kind="ExternalInput")
    Mt = nc.dram_tensor("Mt", [128, NT, NS], BF16, kind="ExternalInput")
    y = nc.dram_tensor("y", [NS, NCLASS], F32, kind="ExternalOutput")

    with TileContext(nc) as tc:
        with (
            tc.tile_pool(name="const", bufs=1) as cp,
            tc.tile_pool(name="mask", bufs=1) as mp,
            tc.tile_pool(name="whext", bufs=1) as wp,
            tc.tile_pool(name="h1g", bufs=1) as gp,
            tc.tile_pool(name="work", bufs=2) as wk,
            tc.tile_pool(name="psA", bufs=1, space="PSUM") as psA,
            tc.tile_pool(name="dram", bufs=1, space="DRAM") as dr,
        ):
            # ---- constants / weights
            ident = cp.tile([128, 128], F32, tag="ident", name="ident")
            make_identity(nc, ident[:])
            ones = cp.tile([1, 128], F32, tag="ones", name="ones")
            nc.vector.memset(ones[:], 1.0)
            ones16 = cp.tile([1, 128], FP16, tag="ones16", name="ones16")
            nc.vector.memset(ones16[:], 1.0)
            wb_t = [cp.tile([128, C1], FP16, tag=f"wb{k}", name=f"wb{k}") for k in range(KX)]
            for k in range(KX):
                nc.sync.dma_start(out=wb_t[k][:], in_=Wb[k * 128:(k + 1) * 128, :])
            bb_t = cp.tile([1, C1], FP16, tag="bb", name="bb")
            nc.sync.dma_start(out=bb_t[:], in_=bb[:])
            wo_t = [cp.tile([128, C2], FP16, tag=f"wo{k}", name=f"wo{k}") for k in range(2)]
            for k in range(2):
                nc.sync.dma_start(out=wo_t[k][:], in_=Wob[k * 128:(k + 1) * 128, :])
            bo_t = cp.tile([1, C2], FP16, tag="bo", name="bo")
            nc.sync.dma_start(out=bo_t[:], in_=bob[:])

            # ---- mask tiles (bf16 additive), resident through both layers
            m_all = mp.tile([128, NT * NS], BF16, tag="mall", name="mall")
            m_t = [m_all[:, j * NS:(j + 1) * NS] for j in range(NT)]

            # ---- phase B: Wh_ext (bias folded in as a K=1 ones-row matmul)
            # whb: fp16 copy for attention lhsT; dstc: f32 dst columns per head
            whb = [wp.tile([128, C1], FP16, tag=f"whb{t}", name=f"whb{t}") for t in range(NT)]
            dstc = [wp.tile([128, NHEADS], F32, tag=f"dstc{t}", name=f"dstc{t}") for t in range(NT)]
            whs = [wp.tile([128, C1], F32, tag=f"whs{t}", name=f"whs{t}") for t in range(4)]
            with tc.tile_pool(name="xpool", bufs=1) as xp:
                torder = [NT, NT + 1, NT + 2, NT + 3] + list(range(NT))
                xchunks = []
                # interleave x chunks with mask loads so neither starves the
                # other in the HWDGE FIFO at kernel start
                mask_plan = {0: (0, 2), 1: (2, 8), 3: (8, 32)}
                for c in range(6):
                    tq0, tq1 = 6 * c, min(6 * (c + 1), NT + 4)
                    xc = xp.tile([128, 6 * NFEAT], FP16, tag="xst", name="xst", bufs=3)
                    nc.sync.dma_start(
                        out=xc[:, 0:(tq1 - tq0) * NFEAT].rearrange(
                            "p (t f) -> p t f", f=NFEAT),
                        in_=xP[tq0:tq1, :, :].rearrange("t p f -> p t f"))
                    xchunks.append(xc)
                    if c in mask_plan:
                        j0, j1 = mask_plan[c]
                        nc.sync.dma_start(out=m_all[:, j0 * NS:j1 * NS],
                                          in_=Mt[:, j0:j1, :])
                for tq, t in enumerate(torder):
                    ps = psA.tile([128, C1], F32, tag="mmps", name="mmps", bufs=2)
                    xk = xchunks[tq // 6][:, (tq % 6) * NFEAT:(tq % 6 + 1) * NFEAT]
                    for k in range(KX):
                        nc.tensor.matmul(out=ps[:], lhsT=xk[:, k * 128:(k + 1) * 128],
                                         rhs=wb_t[k][:],
                                         start=(k == 0), stop=False)
                    nc.tensor.matmul(out=ps[:], lhsT=ones16[:], rhs=bb_t[:],
                                     start=False, stop=True)
                    if t < NT:
                        nc.scalar.copy(out=whb[t][:], in_=ps[:])
                        nc.vector.tensor_copy(
                            out=dstc[t][:], in_=ps[:, 1:1 + 67 * (NHEADS - 1) + 1:67])
                    else:
                        nc.scalar.copy(out=whs[t - NT][:], in_=ps[:])

            # ---- phase C: broadcast src scores across partitions (PE transpose)
            src_b = [cp.tile([128, NS], FP16, tag=f"srcb{h}", name=f"srcb{h}") for h in range(NHEADS)]
            for h in range(NHEADS):
                for t in range(4):
                    tp = psA.tile([128, 128], F32, tag="bc", name="bc", bufs=2)
                    nc.tensor.transpose(
                        out=tp[:], in_=whs[t][:, 67 * h:67 * h + 1].to_broadcast([128, 128]),
                        identity=ident[:])
                    nc.scalar.copy(out=src_b[h][:, t * 128:(t + 1) * 128], in_=tp[:])

            h1t = [cp.tile([128, NS], FP16, tag=f"h1t{i}", name=f"h1t{i}") for i in range(2)]


            def score_tiles(src_tile, dcol_of, whb_of, att, lhs_lo, lhs_hi, slot_base=0):
                """Hot loop over 32 j-tiles: X1 = src+dst (TS), S_l = lrelu (STT on
                DVE), X = S_l + M (TT; mask-add commutes past lrelu), T = exp (ACT,
                paired), then PE accumulates. TS/TT of ~2/3 of tiles go to GPSIMD."""
                for jp in range(NT // 2):
                    sl = wk.tile([128, 2 * NS], FP16, tag="sl", name="sl", bufs=5)
                    for u in range(2):
                        j = 2 * jp + u
                        slot = slot_base + j
                        ett = nc.gpsimd if (slot * 33) % 160 < 33 else nc.vector
                        xt = wk.tile([128, NS], FP16, tag="xt", name="xt", bufs=6)
                        nc.gpsimd.tensor_scalar(out=xt[:], in0=src_tile[:], scalar1=dcol_of(j),
                                                scalar2=None, op0=ALU.add)
                        sl0 = wk.tile([128, NS], FP16, tag="sl0", name="sl0", bufs=6)
                        nc.vector.scalar_tensor_tensor(
                            out=sl0[:], in0=xt[:], scalar=ALPHA,
                            in1=xt[:], op0=ALU.mult, op1=ALU.max)
                        ett.tensor_tensor(out=sl[:, u * NS:(u + 1) * NS], in0=sl0[:],
                                          in1=m_t[j], op=ALU.add)
                    tt = wk.tile([128, 2 * NS], FP16, tag="tt", name="tt", bufs=5)
                    nc.scalar.activation(tt[:], sl[:], AF.Exp)
                    for u in range(2):
                        j = 2 * jp + u
                        nc.tensor.matmul(
                            out=att[:], lhsT=whb_of(j)[:, lhs_lo:lhs_hi],
                            rhs=tt[:, u * NS:(u + 1) * NS],
                            start=(j == 0), stop=(j == NT - 1))

            # ---- phase D: layer-1 attention, one head at a time
            for h in range(NHEADS):
                att = psA.tile([NHID + 1, NS], F32, tag="att", name="att", bufs=2)
                score_tiles(src_b[h], lambda j, h=h: dstc[j][:, h:h + 1],
                            lambda j: whb[j], att, 67 * h + 2, 67 * h + 67,
                            slot_base=h * NT)
                rec = wk.tile([1, NS], F32, tag="rec", name="rec", bufs=2)
                nc.vector.reciprocal(out=rec[:], in_=att[NHID:NHID + 1, :])
                bcn = psA.tile([NHID, NS], F32, tag="bcn", name="bcn", bufs=1)
                nc.tensor.matmul(out=bcn[:], lhsT=ones[:, 0:NHID], rhs=rec[:], start=True, stop=True)
                nsb = wk.tile([NHID, NS], F32, tag="nsb", name="nsb", bufs=2)
                nc.scalar.copy(out=nsb[:], in_=att[0:NHID, :])
                pre = wk.tile([NHID, NS], F32, tag="pre", name="pre", bufs=2)
                nc.vector.tensor_tensor(out=pre[:], in0=nsb[:], in1=bcn[:], op=ALU.mult)
                # elu(x) = min(exp(x),1) - 1 + relu(x)
                ex = wk.tile([NHID, NS], F32, tag="ex", name="ex", bufs=2)
                nc.scalar.activation(ex[:], pre[:], AF.Exp)
                rm1 = wk.tile([NHID, NS], F32, tag="rm1", name="rm1", bufs=2)
                nc.vector.tensor_scalar(out=rm1[:], in0=pre[:], scalar1=0.0, scalar2=-1.0,
                                        op0=ALU.max, op1=ALU.add)
                nc.vector.scalar_tensor_tensor(
                    out=h1t[h // 2][64 * (h % 2):64 * (h % 2) + 64, :],
                    in0=ex[:], scalar=1.0, in1=rm1[:], op0=ALU.min, op1=ALU.add)


            # ---- phase F: Wh2 for own shard only; AllGather the [512, C2]
            # fused result (fp16) instead of gathering h1 itself.
            whs2 = [wp.tile([128, C2], F32, tag=f"whs2_{t}", name=f"whs2_{t}") for t in range(4)]
            whsb = [wp.tile([128, C2], FP16, tag=f"whsb{t}", name=f"whsb{t}") for t in range(4)]
            agin2 = dr.tile([4 * 128, C2], FP16, tag="agin2", name="agin2")
            agout2 = dr.tile([NCORES * 4 * 128, C2], FP16, tag="agout2", name="agout2")
            for t in range(4):
                ps = psA.tile([128, C2], F32, tag="mmps", name="mmps2", bufs=2)
                for k in range(2):
                    nc.tensor.matmul(out=ps[:], lhsT=h1t[k][:, t * 128:(t + 1) * 128],
                                     rhs=wo_t[k][:], start=(k == 0), stop=False)
                nc.tensor.matmul(out=ps[:], lhsT=ones16[:], rhs=bo_t[:],
                                 start=False, stop=True)
                nc.scalar.copy(out=whs2[t][:], in_=ps[:])
                nc.vector.tensor_copy(out=whsb[t][:], in_=ps[:])
                nc.sync.dma_start(out=agin2[t * 128:(t + 1) * 128, :], in_=whsb[t][:])
            if sim_mode:
                for r in range(NCORES):
                    nc.sync.dma_start(out=agout2[r * 512:(r + 1) * 512, :], in_=agin2[:])
            else:
                nc.gpsimd.collective_compute(
                    "AllGather", ALU.bypass,
                    replica_groups=[list(range(NCORES))],
                    ins=[agin2[:].opt()],
                    outs=[agout2[:].opt()])
            wh2all = gp.tile([128, NT * C2], FP16, tag="wh2all", name="wh2all")
            nc.sync.dma_start(
                out=wh2all[:].rearrange("p (q c) -> p q c", c=C2),
                in_=agout2[:].rearrange("(q p) c -> p q c", p=128))
            wh2 = [wh2all[:, t * C2:(t + 1) * C2] for t in range(NT)]
            dst2c = gp.tile([128, NT], F32, tag="dst2c", name="dst2c")
            nc.vector.tensor_copy(out=dst2c[:], in_=wh2all[:, 1:1 + (NT - 1) * C2 + 1:C2])

            # ---- phase G: broadcast src2
            src2_b = cp.tile([128, NS], FP16, tag="src2b", name="src2b")
            for t in range(4):
                tp = psA.tile([128, 128], F32, tag="bc", name="bc", bufs=2)
                nc.tensor.transpose(
                    out=tp[:], in_=whs2[t][:, 0:1].to_broadcast([128, 128]),
                    identity=ident[:])
                nc.scalar.copy(out=src2_b[:, t * 128:(t + 1) * 128], in_=tp[:])

            # ---- phase H: layer-2 attention (single head)
            att2 = psA.tile([65, NS], F32, tag="att", name="att2", bufs=2)
            score_tiles(src2_b, lambda j: dst2c[:, j:j + 1],
                        lambda j: wh2all[:, j * C2:(j + 1) * C2],
                        att2, 2, 67, slot_base=4 * NT)
            rec = wk.tile([1, NS], F32, tag="rec", name="rec2", bufs=2)
            nc.vector.reciprocal(out=rec[:], in_=att2[64:65, :])
            nsb = wk.tile([NCLASS, NS], F32, tag="nsb", name="nsb2", bufs=2)
            nc.scalar.copy(out=nsb[:], in_=att2[0:NCLASS, :])
            for c in range(4):
                tp = psA.tile([128, NCLASS], F32, tag="bc", name="tr", bufs=2)
                nc.tensor.transpose(out=tp[:], in_=nsb[:, c * 128:(c + 1) * 128],
                                    identity=ident[0:NCLASS, 0:NCLASS])
                tpr = psA.tile([128, 1], F32, tag="trr", name="trr", bufs=1)
                nc.tensor.transpose(out=tpr[:], in_=rec[:, c * 128:(c + 1) * 128],
                                    identity=ident[0:1, 0:1])
                rcol = wk.tile([128, 1], F32, tag="rcol", name="rcol", bufs=2)
                nc.vector.tensor_copy(out=rcol[:], in_=tpr[:])
                # o2t = numerator^T * (1/denom), per-partition scalar
                o2t = wk.tile([128, NCLASS], F32, tag="z", name="o2t", bufs=2)
                nc.vector.tensor_scalar(out=o2t[:], in0=tp[:], scalar1=rcol[:, 0:1],
                                        scalar2=None, op0=ALU.mult)
                # elu = min(exp(x),1) - 1 + relu(x)
                exv = wk.tile([128, NCLASS], F32, tag="ez", name="exv", bufs=2)
                nc.scalar.activation(exv[:], o2t[:], AF.Exp)
                rm1 = wk.tile([128, NCLASS], F32, tag="rm1c", name="rm1c", bufs=2)
                nc.vector.tensor_scalar(out=rm1[:], in0=o2t[:], scalar1=0.0, scalar2=-1.0,
                                        op0=ALU.max, op1=ALU.add)
                z = wk.tile([128, NCLASS], F32, tag="zc", name="zc", bufs=2)
                nc.vector.scalar_tensor_tensor(out=z[:], in0=exv[:], scalar=1.0, in1=rm1[:],
                                               op0=ALU.min, op1=ALU.add)
                # log_softmax over classes
                m = wk.tile([128, 1], F32, tag="mred", name="mred", bufs=2)
                nc.vector.reduce_max(out=m[:], in_=z[:], axis=mybir.AxisListType.X)
                negm = wk.tile([128, 1], F32, tag="negm", name="negm", bufs=2)
                nc.vector.tensor_scalar_mul(out=negm[:], in0=m[:], scalar1=-1.0)
                ez = wk.tile([128, NCLASS], F32, tag="ez2", name="ez", bufs=2)
                ssum = wk.tile([128, 1], F32, tag="ssum", name="ssum", bufs=2)
                nc.scalar.activation(ez[:], z[:], AF.Exp, bias=negm[:, 0:1],
                                     accum_out=ssum[:, 0:1])
                ls = wk.tile([128, 1], F32, tag="ls", name="ls", bufs=2)
                nc.scalar.activation(ls[:], ssum[:], AF.Ln)
                mls = wk.tile([128, 1], F32, tag="mls", name="mls", bufs=2)
                nc.vector.tensor_tensor(out=mls[:], in0=m[:], in1=ls[:], op=ALU.add)
                yt = wk.tile([128, NCLASS], F32, tag="yt", name="yt", bufs=2)
                nc.vector.tensor_scalar(out=yt[:], in0=z[:], scalar1=mls[:, 0:1],
                                        scalar2=None, op0=ALU.subtract)
                nc.sync.dma_start(out=y[c * 128:(c + 1) * 128, :], in_=yt[:])

    nc.compile()
    return nc


def _prep_inputs(x, edge_index, W1, b1, a1, ab1, Wo, bo, ao, abo):
    x = np.asarray(x, np.float32)
    W1 = np.asarray(W1, np.float32)
    b1 = np.asarray(b1, np.float32)
    a1 = np.asarray(a1, np.float32)
    ab1 = np.asarray(ab1, np.float32)
    Wo = np.asarray(Wo, np.float32)
    bo = np.asarray(bo, np.float32)
    ao = np.asarray(ao, np.float32)
    abo = np.asarray(abo, np.float32)

    # packed lhsT tiles: xP[t, p, k*128+c] = x[node = t*128+c, feat = k*128+p]
    x16 = x.astype(np.float16)
    xall = x16.reshape(NT, 128, KX, 128).transpose(0, 3, 2, 1).reshape(NT, 128, NFEAT)

    Wb = np.zeros((NFEAT, C1), np.float32)
    bbrow = np.zeros((C1,), np.float32)
    for h in range(NHEADS):
        c = 67 * h
        Wb[:, c + 0] = W1[h] @ a1[h, :NHID]
        Wb[:, c + 1] = W1[h] @ a1[h, NHID:]
        Wb[:, c + 2:c + 2 + NHID] = W1[h]
        bbrow[c + 0] = b1[h] @ a1[h, :NHID]
        bbrow[c + 1] = b1[h] @ a1[h, NHID:] + ab1[h]
        bbrow[c + 2:c + 2 + NHID] = b1[h]
        bbrow[c + 66] = 1.0
    bb = bbrow[None, :].copy()

    Wob = np.zeros((2 * 128, C2), np.float32)
    Wob[:, 0] = Wo @ ao[:NCLASS]
    Wob[:, 1] = Wo @ ao[NCLASS:]
    Wob[:, 2:2 + NCLASS] = Wo
    borow = np.zeros((C2,), np.float32)
    borow[0] = bo @ ao[:NCLASS]
    borow[1] = bo @ ao[NCLASS:] + abo
    borow[2:2 + NCLASS] = bo
    borow[66] = 1.0
    bob = borow[None, :].copy()

    # additive mask, transposed: M[j, i] = 0 if adj[i, j] else -1e4
    ei = np.asarray(edge_index).astype(np.int64)
    Madd = np.full((N, N), MASKVAL, np.float32)
    Madd[ei[1], ei[0]] = 0.0
    Madd = Madd.astype(ml_dtypes.bfloat16)

    in_maps = []
    for c in range(NCORES):
        sl = slice(c * NS, (c + 1) * NS)
        xs = x16[sl].reshape(4, 128, KX, 128).transpose(0, 3, 2, 1).reshape(4, 128, NFEAT)
        in_maps.append({
            "xP": np.ascontiguousarray(np.concatenate([xs, xall], axis=0)),
            "Wb": Wb.astype(np.float16), "bb": bb.astype(np.float16),
            "Wob": Wob.astype(np.float16), "bob": bob.astype(np.float16),
            "Mt": np.ascontiguousarray(
                Madd[:, sl].reshape(NT, 128, NS).transpose(1, 0, 2)),
        })
    return in_maps


def kernel(x, edge_index, W1, b1, a1, ab1, Wo, bo, ao, abo, _trace=False):
    global _compiled
    if _compiled is None:
        _compiled = _build()
    in_maps = _prep_inputs(x, edge_index, W1, b1, a1, ab1, Wo, bo, ao, abo)
    res = run_bass_kernel_spmd(_compiled, in_maps, core_ids=list(range(NCORES)),
                               trace=_trace)
    kernel.last_result = res
    return np.concatenate([res.results[c]["y"] for c in range(NCORES)], axis=0)


# revision 30
# speedup vs baseline: 1132.9620x; 1.0083x over previous
"""Multi-layer GAT (2-layer graph attention network) on 8 Trainium2 NeuronCores.

Sharding: query-node rows of the NxN attention problem are sharded across the
8 cores (512 rows each); weights and the column copy of Wh are replicated.
The only collective is an AllGather of the layer-1 output h [4096, 256]
(transposed, f-major) between the two GAT layers.

Score math per core, computed transposed (j on partitions, i on free dim):
    X[j, i]  = src[i] + dst[j] + M[j, i]          (M = 0 for edge, -1e4 else)
    T[j, i]  = exp(max(X, 0.2 X))                 (exact leaky_relu via DVE)
    out[f,i] = (Wh^T @ T)[f, i] / (ones @ T)[i]   (ones column fused in lhsT)
src/dst/bias terms are folded into an extended weight matrix on the host, so
they fall out of the same matmul that computes Wh.
"""

import numpy as np
import ml_dtypes

import concourse.bacc as bacc
import concourse.mybir as mybir
from concourse.tile import TileContext
from concourse.bass_utils import run_bass_kernel_spmd
from concourse.masks import make_identity

F32 = mybir.dt.float32
BF16 = mybir.dt.bfloat16
FP16 = mybir.dt.float16
F32R = mybir.dt.float32r
AF = mybir.ActivationFunctionType
ALU = mybir.AluOpType

N, NFEAT, NHID, NCLASS, NHEADS = 4096, 512, 64, 40, 4
NCORES = 8
NS = N // NCORES          # 512 rows (query nodes) per core
NT = N // 128             # 32 j-tiles
KX = NFEAT // 128         # 4 k-tiles over input features
C1 = NHEADS * (NHID + 3)  # 268 fused-weight cols, 67 per head: src,dst,Wh[64],ones
C2 = 67                   # src2,dst2,Wo(40)+pad(24),ones — ones at col 66 so denom hits partition 64
ALPHA = 0.2
MASKVAL = -1e4

_compiled = None


def _build(sim_mode=False, stop_after=None):
    nc = bacc.Bacc("TRN2", num_devices=1 if sim_mode else NCORES)

    xP = nc.dram_tensor("xP", [NT + 4, 128, NFEAT], FP16, kind="ExternalInput")
    Wb = nc.dram_tensor("Wb", [NFEAT, C1], FP16, kind="ExternalInput")
    bb = nc.dram_tensor("bb", [1, C1], FP16, kind="ExternalInput")
    Wob = nc.dram_tensor("Wob", [2 * 128, C2], FP16, kind="ExternalInput")
    bob = nc.dram_tensor("bob", [1, C2], FP16, kind="ExternalInput")
    Mt = nc.dram_tensor("Mt", [128, NT, NS], BF16, kind="ExternalInput")
    y = nc.dram_tensor("y", [NS, NCLASS], F32, kind="ExternalOutput")

    with TileContext(nc) as tc:
        with (
            tc.tile_pool(name="const", bufs=1) as cp,
            tc.tile_pool(name="mask", bufs=1) as mp,
            tc.tile_pool(name="whext", bufs=1) as wp,
            tc.tile_pool(name="h1g", bufs=1) as gp,
            tc.tile_pool(name="work", bufs=2) as wk,
            tc.tile_pool(name="psA", bufs=1, space="PSUM") as psA,
            tc.tile_pool(name="dram", bufs=1, space="DRAM") as dr,
        ):
            # ---- constants / weights
            ident = cp.tile([128, 128], F32, tag="ident", name="ident")
            make_identity(nc, ident[:])
            ones = cp.tile([1, 128], F32, tag="ones", name="ones")
            nc.vector.memset(ones[:], 1.0)
            ones16 = cp.tile([1, 128], FP16, tag="ones16", name="ones16")
            nc.vector.memset(ones16[:], 1.0)
            wb_t = [cp.tile([128, C1], FP16, tag=f"wb{k}", name=f"wb{k}") for k in range(KX)]
            for k in range(KX):
                nc.sync.dma_start(out=wb_t[k][:], in_=Wb[k * 128:(k + 1) * 128, :])
            bb_t = cp.tile([1, C1], FP16, tag="bb", name="bb")
            nc.sync.dma_start(out=bb_t[:], in_=bb[:])
            wo_t = [cp.tile([128, C2], FP16, tag=f"wo{k}", name=f"wo{k}") for k in range(2)]
            for k in range(2):
                nc.sync.dma_start(out=wo_t[k][:], in_=Wob[k * 128:(k + 1) * 128, :])
            bo_t = cp.tile([1, C2], FP16, tag="bo", name="bo")
            nc.sync.dma_start(out=bo_t[:], in_=bob[:])

            # ---- mask tiles (bf16 additive), resident through both layers
            m_all = mp.tile([128, NT * NS], BF16, tag="mall", name="mall")
            m_t = [m_all[:, j * NS:(j + 1) * NS] for j in range(NT)]

            # ---- phase B: Wh_ext (bias folded in as a K=1 ones-row matmul)
            # whb: fp16 copy for attention lhsT; dstc: f32 dst columns per head
            whb = [wp.tile([128, C1], FP16, tag=f"whb{t}", name=f"whb{t}") for t in range(NT)]
            dstc = [wp.tile([128, NHEADS], F32, tag=f"dstc{t}", name=f"dstc{t}") for t in range(NT)]
            whs = [wp.tile([128, C1], F32, tag=f"whs{t}", name=f"whs{t}") for t in range(4)]
            with tc.tile_pool(name="xpool", bufs=1) as xp:
                torder = [NT, NT + 1, NT + 2, NT + 3] + list(range(NT))
                xchunks = []
                # interleave x chunks with mask loads so neither starves the
                # other in the HWDGE FIFO at kernel start
                mask_plan = {0: (0, 2), 1: (2, 8), 3: (8, 32)}
                for c in range(6):
                    tq0, tq1 = 6 * c, min(6 * (c + 1), NT + 4)
                    xc = xp.tile([128, 6 * NFEAT], FP16, tag="xst", name="xst", bufs=3)
                    nc.sync.dma_start(
                        out=xc[:, 0:(tq1 - tq0) * NFEAT].rearrange(
                            "p (t f) -> p t f", f=NFEAT),
                        in_=xP[tq0:tq1, :, :].rearrange("t p f -> p t f"))
                    xchunks.append(xc)
                    if c in mask_plan:
                        j0, j1 = mask_plan[c]
                        nc.sync.dma_start(out=m_all[:, j0 * NS:j1 * NS],
                                          in_=Mt[:, j0:j1, :])
                for tq, t in enumerate(torder):
                    ps = psA.tile([128, C1], F32, tag="mmps", name="mmps", bufs=2)
                    xk = xchunks[tq // 6][:, (tq % 6) * NFEAT:(tq % 6 + 1) * NFEAT]
                    for k in range(KX):
                        nc.tensor.matmul(out=ps[:], lhsT=xk[:, k * 128:(k + 1) * 128],
                                         rhs=wb_t[k][:],
                                         start=(k == 0), stop=False)
                    nc.tensor.matmul(out=ps[:], lhsT=ones16[:], rhs=bb_t[:],
                                     start=False, stop=True)
                    if t < NT:
                        nc.scalar.copy(out=whb[t][:], in_=ps[:])
                        nc.scalar.copy(
                            out=dstc[t][:], in_=ps[:, 1:1 + 67 * (NHEADS - 1) + 1:67])
                    else:
                        nc.scalar.copy(out=whs[t - NT][:], in_=ps[:])

            # ---- phase C: broadcast src scores across partitions (PE transpose)
            src_b = [cp.tile([128, NS], FP16, tag=f"srcb{h}", name=f"srcb{h}") for h in range(NHEADS)]
            for h in range(NHEADS):
                for t in range(4):
                    tp = psA.tile([128, 128], F32, tag="bc", name="bc", bufs=2)
                    nc.tensor.transpose(
                        out=tp[:], in_=whs[t][:, 67 * h:67 * h + 1].to_broadcast([128, 128]),
                        identity=ident[:])
                    nc.scalar.copy(out=src_b[h][:, t * 128:(t + 1) * 128], in_=tp[:])

            h1t = [cp.tile([128, NS], FP16, tag=f"h1t{i}", name=f"h1t{i}") for i in range(2)]


            def score_tiles(src_tile, dcol_of, whb_of, att, lhs_lo, lhs_hi, slot_base=0):
                """Hot loop over 32 j-tiles: X1 = src+dst (TS), S_l = lrelu (STT on
                DVE), X = S_l + M (TT; mask-add commutes past lrelu), T = exp (ACT,
                paired), then PE accumulates. TS/TT of ~2/3 of tiles go to GPSIMD."""
                for jp in range(NT // 2):
                    sl = wk.tile([128, 2 * NS], FP16, tag="sl", name="sl", bufs=5)
                    for u in range(2):
                        j = 2 * jp + u
                        slot = slot_base + j
                        ett = nc.gpsimd if (slot * 25) % 160 < 25 else nc.vector
                        xt = wk.tile([128, NS], FP16, tag="xt", name="xt", bufs=6)
                        nc.gpsimd.tensor_scalar(out=xt[:], in0=src_tile[:], scalar1=dcol_of(j),
                                                scalar2=None, op0=ALU.add)
                        sl0 = wk.tile([128, NS], FP16, tag="sl0", name="sl0", bufs=6)
                        nc.vector.scalar_tensor_tensor(
                            out=sl0[:], in0=xt[:], scalar=ALPHA,
                            in1=xt[:], op0=ALU.mult, op1=ALU.max)
                        ett.tensor_tensor(out=sl[:, u * NS:(u + 1) * NS], in0=sl0[:],
                                          in1=m_t[j], op=ALU.add)
                    tt = wk.tile([128, 2 * NS], FP16, tag="tt", name="tt", bufs=5)
                    nc.scalar.activation(tt[:], sl[:], AF.Exp)
                    for u in range(2):
                        j = 2 * jp + u
                        nc.tensor.matmul(
                            out=att[:], lhsT=whb_of(j)[:, lhs_lo:lhs_hi],
                            rhs=tt[:, u * NS:(u + 1) * NS],
                            start=(j == 0), stop=(j == NT - 1))

            # ---- phase D: layer-1 attention, one head at a time
            for h in range(NHEADS):
                att = psA.tile([NHID + 1, NS], F32, tag="att", name="att", bufs=2)
                score_tiles(src_b[h], lambda j, h=h: dstc[j][:, h:h + 1],
                            lambda j: whb[j], att, 67 * h + 2, 67 * h + 67,
                            slot_base=h * NT)
                rec = wk.tile([1, NS], F32, tag="rec", name="rec", bufs=2)
                nc.vector.reciprocal(out=rec[:], in_=att[NHID:NHID + 1, :])
                bcn = psA.tile([NHID, NS], F32, tag="bcn", name="bcn", bufs=1)
                nc.tensor.matmul(out=bcn[:], lhsT=ones[:, 0:NHID], rhs=rec[:], start=True, stop=True)
                nsb = wk.tile([NHID, NS], F32, tag="nsb", name="nsb", bufs=2)
                nc.scalar.copy(out=nsb[:], in_=att[0:NHID, :])
                pre = wk.tile([NHID, NS], F32, tag="pre", name="pre", bufs=2)
                nc.vector.tensor_tensor(out=pre[:], in0=nsb[:], in1=bcn[:], op=ALU.mult)
                # elu(x) = min(exp(x),1) - 1 + relu(x)
                ex = wk.tile([NHID, NS], F32, tag="ex", name="ex", bufs=2)
                nc.scalar.activation(ex[:], pre[:], AF.Exp)
                rm1 = wk.tile([NHID, NS], F32, tag="rm1", name="rm1", bufs=2)
                nc.vector.tensor_scalar(out=rm1[:], in0=pre[:], scalar1=0.0, scalar2=-1.0,
                                        op0=ALU.max, op1=ALU.add)
                nc.vector.scalar_tensor_tensor(
                    out=h1t[h // 2][64 * (h % 2):64 * (h % 2) + 64, :],
                    in0=ex[:], scalar=1.0, in1=rm1[:], op0=ALU.min, op1=ALU.add)


            # ---- phase F: Wh2 for own shard only; AllGather the [512, C2]
            # fused result (fp16) instead of gathering h1 itself.
            whs2 = [wp.tile([128, C2], F32, tag=f"whs2_{t}", name=f"whs2_{t}") for t in range(4)]
            whsb = [wp.tile([128, C2], FP16, tag=f"whsb{t}", name=f"whsb{t}") for t in range(4)]
            agin2 = dr.tile([4 * 128, C2], FP16, tag="agin2", name="agin2")
            agout2 = dr.tile([NCORES * 4 * 128, C2], FP16, tag="agout2", name="agout2")
            for t in range(4):
                ps = psA.tile([128, C2], F32, tag="mmps", name="mmps2", bufs=2)
                for k in range(2):
                    nc.tensor.matmul(out=ps[:], lhsT=h1t[k][:, t * 128:(t + 1) * 128],
                                     rhs=wo_t[k][:], start=(k == 0), stop=False)
                nc.tensor.matmul(out=ps[:], lhsT=ones16[:], rhs=bo_t[:],
                                 start=False, stop=True)
                nc.scalar.copy(out=whs2[t][:], in_=ps[:])
                nc.vector.tensor_copy(out=whsb[t][:], in_=ps[:])
                nc.sync.dma_start(out=agin2[t * 128:(t + 1) * 128, :], in_=whsb[t][:])
            if sim_mode:
                for r in range(NCORES):
                    nc.sync.dma_start(out=agout2[r * 512:(r + 1) * 512, :], in_=agin2[:])
            else:
                nc.gpsimd.collective_compute(
                    "AllGather", ALU.bypass,
                    replica_groups=[list(range(NCORES))],
                    ins=[agin2[:].opt()],
                    outs=[agout2[:].opt()])
            wh2all = gp.tile([128, NT * C2], FP16, tag="wh2all", name="wh2all")
            nc.sync.dma_start(
                out=wh2all[:].rearrange("p (q c) -> p q c", c=C2),
                in_=agout2[:].rearrange("(q p) c -> p q c", p=128))
            wh2 = [wh2all[:, t * C2:(t + 1) * C2] for t in range(NT)]
            dst2c = gp.tile([128, NT], F32, tag="dst2c", name="dst2c")
            nc.scalar.copy(out=dst2c[:], in_=wh2all[:, 1:1 + (NT - 1) * C2 + 1:C2])

            # ---- phase G: broadcast src2
            src2_b = cp.tile([128, NS], FP16, tag="src2b", name="src2b")
            for t in range(4):
                tp = psA.tile([128, 128], F32, tag="bc", name="bc", bufs=2)
                nc.tensor.transpose(
                    out=tp[:], in_=whs2[t][:, 0:1].to_broadcast([128, 128]),
                    identity=ident[:])
                nc.scalar.copy(out=src2_b[:, t * 128:(t + 1) * 128], in_=tp[:])

            # ---- phase H: layer-2 attention (single head)
            att2 = psA.tile([65, NS], F32, tag="att", name="att2", bufs=2)
            score_tiles(src2_b, lambda j: dst2c[:, j:j + 1],
                        lambda j: wh2all[:, j * C2:(j + 1) * C2],
                        att2, 2, 67, slot_base=4 * NT)
            rec = wk.tile([1, NS], F32, tag="rec", name="rec2", bufs=2)
            nc.vector.reciprocal(out=rec[:], in_=att2[64:65, :])
            nsb = wk.tile([NCLASS, NS], F32, tag="nsb", name="nsb2", bufs=2)
            nc.scalar.copy(out=nsb[:], in_=att2[0:NCLASS, :])
            for c in range(4):
                tp = psA.tile([128, NCLASS], F32, tag="bc", name="tr", bufs=2)
                nc.tensor.transpose(out=tp[:], in_=nsb[:, c * 128:(c + 1) * 128],
                                    identity=ident[0:NCLASS, 0:NCLASS])
                tpr = psA.tile([128, 1], F32, tag="trr", name="trr", bufs=1)
                nc.tensor.transpose(out=tpr[:], in_=rec[:, c * 128:(c + 1) * 128],
                                    identity=ident[0:1, 0:1])
                rcol = wk.tile([128, 1], F32, tag="rcol", name="rcol", bufs=2)
                nc.vector.tensor_copy(out=rcol[:], in_=tpr[:])
                # o2t = numerator^T * (1/denom), per-partition scalar
                o2t = wk.tile([128, NCLASS], F32, tag="z", name="o2t", bufs=2)
                nc.vector.tensor_scalar(out=o2t[:], in0=tp[:], scalar1=rcol[:, 0:1],
                                        scalar2=None, op0=ALU.mult)
                # elu = min(exp(x),1) - 1 + relu(x)
                exv = wk.tile([128, NCLASS], F32, tag="ez", name="exv", bufs=2)
                nc.scalar.activation(exv[:], o2t[:], AF.Exp)
                rm1 = wk.tile([128, NCLASS], F32, tag="rm1c", name="rm1c", bufs=2)
                nc.vector.tensor_scalar(out=rm1[:], in0=o2t[:], scalar1=0.0, scalar2=-1.0,
                                        op0=ALU.max, op1=ALU.add)
                z = wk.tile([128, NCLASS], F32, tag="zc", name="zc", bufs=2)
                nc.vector.scalar_tensor_tensor(out=z[:], in0=exv[:], scalar=1.0, in1=rm1[:],
                                               op0=ALU.min, op1=ALU.add)
                # log_softmax over classes
                m = wk.tile([128, 1], F32, tag="mred", name="mred", bufs=2)
                nc.vector.reduce_max(out=m[:], in_=z[:], axis=mybir.AxisListType.X)
                negm = wk.tile([128, 1], F32, tag="negm", name="negm", bufs=2)
                nc.vector.tensor_scalar_mul(out=negm[:], in0=m[:], scalar1=-1.0)
                ez = wk.tile([128, NCLASS], F32, tag="ez2", name="ez", bufs=2)
                ssum = wk.tile([128, 1], F32, tag="ssum", name="ssum", bufs=2)
                nc.scalar.activation(ez[:], z[:], AF.Exp, bias=negm[:, 0:1],
                                     accum_out=ssum[:, 0:1])
                ls = wk.tile([128, 1], F32, tag="ls", name="ls", bufs=2)
                nc.scalar.activation(ls[:], ssum[:], AF.Ln)
                mls = wk.tile([128, 1], F32, tag="mls", name="mls", bufs=2)
                nc.vector.tensor_tensor(out=mls[:], in0=m[:], in1=ls[:], op=ALU.add)
                yt = wk.tile([128, NCLASS], F32, tag="yt", name="yt", bufs=2)
                nc.vector.tensor_scalar(out=yt[:], in0=z[:], scalar1=mls[:, 0:1],
                                        scalar2=None, op0=ALU.subtract)
                nc.sync.dma_start(out=y[c * 128:(c + 1) * 128, :], in_=yt[:])

    nc.compile()
    return nc


def _prep_inputs(x, edge_index, W1, b1, a1, ab1, Wo, bo, ao, abo):
    x = np.asarray(x, np.float32)
    W1 = np.asarray(W1, np.float32)
    b1 = np.asarray(b1, np.float32)
    a1 = np.asarray(a1, np.float32)
    ab1 = np.asarray(ab1, np.float32)
    Wo = np.asarray(Wo, np.float32)
    bo = np.asarray(bo, np.float32)
    ao = np.asarray(ao, np.float32)
    abo = np.asarray(abo, np.float32)

    # packed lhsT tiles: xP[t, p, k*128+c] = x[node = t*128+c, feat = k*128+p]
    x16 = x.astype(np.float16)
    xall = x16.reshape(NT, 128, KX, 128).transpose(0, 3, 2, 1).reshape(NT, 128, NFEAT)

    Wb = np.zeros((NFEAT, C1), np.float32)
    bbrow = np.zeros((C1,), np.float32)
    for h in range(NHEADS):
        c = 67 * h
        Wb[:, c + 0] = W1[h] @ a1[h, :NHID]
        Wb[:, c + 1] = W1[h] @ a1[h, NHID:]
        Wb[:, c + 2:c + 2 + NHID] = W1[h]
        bbrow[c + 0] = b1[h] @ a1[h, :NHID]
        bbrow[c + 1] = b1[h] @ a1[h, NHID:] + ab1[h]
        bbrow[c + 2:c + 2 + NHID] = b1[h]
        bbrow[c + 66] = 1.0
    bb = bbrow[None, :].copy()

    Wob = np.zeros((2 * 128, C2), np.float32)
    Wob[:, 0] = Wo @ ao[:NCLASS]
    Wob[:, 1] = Wo @ ao[NCLASS:]
    Wob[:, 2:2 + NCLASS] = Wo
    borow = np.zeros((C2,), np.float32)
    borow[0] = bo @ ao[:NCLASS]
    borow[1] = bo @ ao[NCLASS:] + abo
    borow[2:2 + NCLASS] = bo
    borow[66] = 1.0
    bob = borow[None, :].copy()

    # additive mask, transposed: M[j, i] = 0 if adj[i, j] else -1e4
    ei = np.asarray(edge_index).astype(np.int64)
    Madd = np.full((N, N), MASKVAL, np.float32)
    Madd[ei[1], ei[0]] = 0.0
    Madd = Madd.astype(ml_dtypes.bfloat16)

    in_maps = []
    for c in range(NCORES):
        sl = slice(c * NS, (c + 1) * NS)
        xs = x16[sl].reshape(4, 128, KX, 128).transpose(0, 3, 2, 1).reshape(4, 128, NFEAT)
        in_maps.append({
            "xP": np.ascontiguousarray(np.concatenate([xs, xall], axis=0)),
            "Wb": Wb.astype(np.float16), "bb": bb.astype(np.float16),
            "Wob": Wob.astype(np.float16), "bob": bob.astype(np.float16),
            "Mt": np.ascontiguousarray(
                Madd[:, sl].reshape(NT, 128, NS).transpose(1, 0, 2)),
        })
    return in_maps


def kernel(x, edge_index, W1, b1, a1, ab1, Wo, bo, ao, abo, _trace=False):
    global _compiled
    if _compiled is None:
        _compiled = _build()
    in_maps = _prep_inputs(x, edge_index, W1, b1, a1, ab1, Wo, bo, ao, abo)
    res = run_bass_kernel_spmd(_compiled, in_maps, core_ids=list(range(NCORES)),
                               trace=_trace)
    kernel.last_result = res
    return np.concatenate([res.results[c]["y"] for c in range(NCORES)], axis=0)
